# revision 17
# baseline (speedup 1.0000x reference)
"""Trainium2 Bass kernel for single-head causal attention with projections.

Reference computation (B=4, T=4096, D=1024, H=64):
    qh = q @ Wq; kh = k @ Wk; vh = v @ Wv          # [B,T,H]
    S  = qh @ kh.T / sqrt(H)  (causal masked)       # [B,T,T]
    out = softmax(S) @ vh                           # [B,T,H]

Sharding: 8 cores = 4 batches x 2 query-halves. Each core owns one batch's
full K/V and 8 query tiles of 256 rows, chosen by folded pairing so causal
work is balanced; a position-padded schedule makes all 8 cores run one
identical SPMD program (per-core differences live entirely in the data).

v6 design notes:
  - bf16 everywhere on-chip; scores stay in ST orientation (tk on
    partitions): exp(S^T) feeds the PV matmul directly and an appended ones
    column in vh gives the softmax denominator for free.  No running max:
    scores are O(5), exp is safely in fp32 range.
  - All input DMAs are 2D [128, 1024] APs on ONE queue in need-order
    (q, then k/v per wave) so the critical path is never bandwidth-shared.
  - PE 32x32 tiling packs pairs of matmuls to run concurrently:
    k-proj (cols 0-63) || v-proj (cols 64-127) halve projection time, and
    scores for chunks (m, m+1) run as row-tiles (rows 0-63 / 64-127)
    against khT/qhT copies duplicated onto partitions 64-127 (SBUF-to-SBUF
    DMA, no HBM cost).  Attention runs as 4 per-pair passes over the
    resident khT/vh1, one [128,1024] double score tile + one exp per chunk
    pair (40 ACTIVATEs total).
  - Projection matmuls of the NEXT kv group and lazy pair finalization are
    interleaved as PE filler between attention units so the in-order PE
    never idles and the HAM clock-gate keeps the array at 2.4 GHz.
  - Causal tail masks collapse to 4 shared [128,256] blocks per core (the
    block depends only on 128*s - 256*half), applied on DVE.
"""

import collections

import numpy as np

B, T, D, H = 4, 4096, 1024, 64
TILE = 256          # tq position tile
GROUP = 512         # kv / projection t-group
NPOS = 8            # q position tiles per core
DC = D // 128       # d chunks
NKV = T // 128      # kv chunks
NG = T // GROUP     # kv groups (8)
TQ = NPOS * TILE    # q rows per core (2048)
QG = TQ // GROUP    # q groups (4)
NPAIR = NPOS // 2   # position pairs (4)

# per-position kv chunk counts (identical across cores): 32,28,...,4
COUNTS = [NKV - 4 * p for p in range(NPOS)]
# tile indices owned by a core: half 0 -> even tiles, half 1 -> odd tiles
TILES_H0 = [14 - 2 * p for p in range(NPOS)]
TILES_H1 = [15 - 2 * p for p in range(NPOS)]

_CACHE = {}


def _build_program(counts, apply_tail):
    import concourse.bacc as bacc
    import concourse.mybir as mybir
    import concourse.tile as tile
    from concourse.masks import make_identity

    f32 = mybir.dt.float32
    bf16 = mybir.dt.bfloat16

    nc = bacc.Bacc(None, target_bir_lowering=False, debug=False)
    qT = nc.declare_dram_parameter("qT", [D, TQ], bf16, isOutput=False)
    kT = nc.declare_dram_parameter("kT", [D, T], bf16, isOutput=False)
    vT = nc.declare_dram_parameter("vT", [D, T], bf16, isOutput=False)
    # packed [128, (3 tensors x 8 chunks), H]: partition-contiguous layout
    wall = nc.declare_dram_parameter("wall", [128, 3 * DC, H], bf16,
                                     isOutput=False)
    if apply_tail:
        tmask = nc.declare_dram_parameter(
            "tmask", [128, 4, TILE], bf16, isOutput=False)
    out = nc.declare_dram_parameter("out", [TQ, H], f32, isOutput=True)

    qT_r = qT.rearrange("(c p) t -> p c t", p=128)
    kT_r = kT.rearrange("(c p) t -> p c t", p=128)
    vT_r = vT.rearrange("(c p) t -> p c t", p=128)
    out_r = out.rearrange("(s p) h -> p s h", p=128)
    scale = 1.0 / float(np.sqrt(H))

    with tile.TileContext(nc) as tc:
        with (
            tc.tile_pool(name="singles", bufs=1) as singles,
            tc.tile_pool(name="qstage", bufs=1) as qstage,
            tc.tile_pool(name="kring", bufs=4) as kring,
            tc.tile_pool(name="vring", bufs=4) as vring,
            tc.tile_pool(name="work", bufs=6) as work,
            tc.tile_pool(name="small", bufs=3) as small,
            tc.tile_pool(name="proj_ps", bufs=2, space="PSUM") as pps,
            tc.tile_pool(name="st_ps", bufs=2, space="PSUM") as stps,
            tc.tile_pool(name="pvt_ps", bufs=1, space="PSUM") as pvtps,
        ):
            # ---- constants / staging (DMA issue order matters) ----
            w_sb = singles.tile([128, 3 * DC, H], bf16, tag="wall")
            nc.sync.dma_start(out=w_sb, in_=wall[:, :, :])
            wq_sb = w_sb[:, 0 * DC:1 * DC, :]
            wk_sb = w_sb[:, 1 * DC:2 * DC, :]
            wv_sb = w_sb[:, 2 * DC:3 * DC, :]
            tm_sb = None
            if apply_tail:
                tm_sb = singles.tile([128, 4, TILE], bf16, tag="tm")
                nc.sync.dma_start(out=tm_sb, in_=tmask[:, :, :])

            # qT resident; 2D DMAs per (chunk, half); half 1 is issued
            # after kv wave 0 (sweep A only needs half 0 early)
            qT_sb = qstage.tile([128, DC, TQ], bf16, tag="qt")

            def emit_q_dma(hh):
                sl = slice(hh * TQ // 2, (hh + 1) * TQ // 2)
                for c in range(DC):
                    nc.sync.dma_start(out=qT_sb[:, c, sl],
                                      in_=qT_r[:, c, sl])

            emit_q_dma(1)

            ident_b = singles.tile([H, H], bf16, tag="idb")
            make_identity(nc, ident_b)
            ident_f = singles.tile([H + 1, H + 1], f32, tag="idf")
            make_identity(nc, ident_f)
            # identity copy on partitions 64-127 for hi-half transposes
            ident_hi = singles.tile([128, H], bf16, tag="idh")
            nc.gpsimd.dma_start(out=ident_hi[H:128, :], in_=ident_b)

            qhT = singles.tile([H, TQ], bf16, tag="qhT")
            khT = singles.tile([H, T], bf16, tag="khT")
            vh1 = singles.tile([128, NKV, H + 1], bf16, tag="vh1")
            nc.vector.memset(vh1[:, :, H:H + 1], 1.0)
            out_sb = singles.tile([128, TQ // 128, H], f32, tag="osb")

            kv_tiles = {}

            def emit_wave_dma(w):
                ksl = slice(w * 2 * GROUP, (w + 1) * 2 * GROUP)
                kt = kring.tile([128, DC, 2 * GROUP], bf16, tag="kt")
                vt = vring.tile([128, DC, 2 * GROUP], bf16, tag="vt")
                for c in range(DC):
                    nc.sync.dma_start(out=kt[:, c, :], in_=kT_r[:, c, ksl])
                    nc.sync.dma_start(out=vt[:, c, :], in_=vT_r[:, c, ksl])
                kv_tiles[w] = (kt, vt)

            emit_wave_dma(0)
            emit_q_dma(0)
            emit_wave_dma(1)

            # ---- q projection (col-tiled pairs); duo 0 upfront, duo 1
            # deferred into the filler queue (sweep A only needs cols 0-1024)
            def qproj_piece(qp, cs, ce, st):
                g0, g1 = 2 * qp, 2 * qp + 1
                s0 = slice(g0 * GROUP, (g0 + 1) * GROUP)
                s1 = slice(g1 * GROUP, (g1 + 1) * GROUP)
                if cs == 0:
                    st["pha"] = pps.tile([128, GROUP], f32, tag="ph",
                                         name="phqa")
                    st["phb"] = pps.tile([128, GROUP], f32, tag="ph",
                                         name="phqb")
                for c in range(cs, ce):
                    nc.tensor.matmul(st["pha"][0:H, :], wq_sb[:, c, :],
                                     qT_sb[:, c, s0],
                                     start=(c == 0), stop=(c == DC - 1),
                                     tile_position=(0, 0))
                    nc.tensor.matmul(st["phb"][H:128, :], wq_sb[:, c, :],
                                     qT_sb[:, c, s1],
                                     start=(c == 0), stop=(c == DC - 1),
                                     tile_position=(0, 64))
                if ce == DC:
                    nc.vector.tensor_copy(qhT[:, s0], st["pha"][0:H, :])
                    nc.vector.tensor_copy(qhT[:, s1], st["phb"][H:128, :])

            stq1 = {}
            for cs in range(0, DC, 2):
                qproj_piece(1, cs, cs + 2, stq1)

            # ---- filler machinery: small PE pieces run between attn units --
            fill_q = collections.deque()

            def drain_fill(n):
                for _ in range(min(n, len(fill_q))):
                    fill_q.popleft()()

            def push_proj_group(g):
                w, half = divmod(g, 2)
                kt, vt = kv_tiles[w]
                hsl = slice(half * GROUP, (half + 1) * GROUP)
                gsl = slice(g * GROUP, (g + 1) * GROUP)
                st = {}

                def kvproj(cs, ce):
                    def f():
                        if cs == 0:
                            st["phk"] = pps.tile([128, GROUP], f32, tag="ph",
                                                 name="phk")
                            st["phv"] = pps.tile([128, GROUP], f32, tag="ph",
                                                 name="phv")
                        for c in range(cs, ce):
                            nc.tensor.matmul(
                                st["phk"][0:H, :], wk_sb[:, c, :],
                                kt[:, c, hsl],
                                start=(c == 0), stop=(c == DC - 1),
                                tile_position=(0, 0))
                            nc.tensor.matmul(
                                st["phv"][H:128, :], wv_sb[:, c, :],
                                vt[:, c, hsl],
                                start=(c == 0), stop=(c == DC - 1),
                                tile_position=(0, 64))
                        if ce == DC:
                            nc.vector.tensor_copy(khT[:, gsl],
                                                  st["phk"][0:H, :])
                            st["vtmp"] = small.tile(
                                [128, GROUP], bf16, tag="vtmp", name="vtmp")
                            nc.vector.tensor_copy(st["vtmp"][H:128, :],
                                                  st["phv"][H:128, :])
                    return f

                def vtrans(s):
                    def f():
                        tr = pps.tile([128, H], bf16, tag="ph")
                        nc.tensor.transpose(
                            tr, st["vtmp"][H:128, s * 128:(s + 1) * 128],
                            ident_hi[H:128, :], tile_position=(64, 0))
                        nc.vector.tensor_copy(vh1[:, g * 4 + s, 0:H], tr)
                    return f

                for cs in range(0, DC, 2):
                    fill_q.append(kvproj(cs, cs + 2))
                for s in range(GROUP // 128):
                    fill_q.append(vtrans(s))

            def push_finalize(j):
                st = {}

                def fcopy():
                    st["pvt_sb"] = small.tile([H + 1, 2 * TILE], f32,
                                              tag="pvtsb", name="pvtsb")
                    nc.vector.tensor_copy(st["pvt_sb"], pvt[:, j % 2, :])

                def fnorm(s):
                    def f():
                        tr = pps.tile([128, H + 1], f32, tag="ph")
                        nc.tensor.transpose(
                            tr, st["pvt_sb"][:, s * 128:(s + 1) * 128],
                            ident_f)
                        ofull = small.tile([128, H + 1], f32, tag="of")
                        nc.vector.tensor_copy(ofull, tr)
                        rec = small.tile([128, 1], f32, tag="rec")
                        nc.vector.reciprocal(rec, ofull[:, H:H + 1])
                        nc.vector.tensor_scalar_mul(
                            out_sb[:, j * 4 + s, :], ofull[:, :H], rec)
                    return f

                def fout():
                    nc.gpsimd.dma_start(
                        out=out_r[:, j * 4:(j + 1) * 4, :],
                        in_=out_sb[:, j * 4:(j + 1) * 4, :])

                fill_q.append(fcopy)
                for s in range(2 * TILE // 128):
                    fill_q.append(fnorm(s))
                fill_q.append(fout)

            # ---- attention: 4 per-pair passes over resident khT/vh1 ----
            pvt = pvtps.tile([H + 1, 2, 2 * TILE], f32, tag="pvt")
            pending = []

            def flush_pending(limit):
                while len(pending) > limit:
                    m, j, width, psb, off = pending.pop(0)
                    last = m == counts[2 * j] - 1
                    nc.tensor.matmul(
                        pvt[:, j % 2, :width], vh1[:, m, :],
                        psb[:, off:off + width],
                        start=(m == 0), stop=last,
                        skip_group_check=True)
                    if last:
                        push_finalize(j)

            def emit_unit(duo, m):
                jA, jB = 2 * duo, 2 * duo + 1
                wA = 0
                if counts[2 * jA] > m:
                    wA = 2 * TILE if counts[2 * jA + 1] > m else TILE
                wB = 0
                if counts[2 * jB] > m:
                    wB = 2 * TILE if counts[2 * jB + 1] > m else TILE
                if wA == 0 and wB == 0:
                    return
                kh = khT[:, m * 128:(m + 1) * 128]
                stp = stps.tile([128, 4 * TILE], f32, tag="st")
                if wA:
                    nc.tensor.matmul(
                        stp[:, :wA], kh,
                        qhT[:, jA * 2 * TILE:jA * 2 * TILE + wA],
                        start=True, stop=True)
                if wB:
                    nc.tensor.matmul(
                        stp[:, 2 * TILE:2 * TILE + wB], kh,
                        qhT[:, jB * 2 * TILE:jB * 2 * TILE + wB],
                        start=True, stop=True)
                ew = (2 * TILE + wB) if wB else wA
                psb = work.tile([128, 4 * TILE], bf16, tag="p")
                nc.scalar.activation(
                    psb[:, :ew], stp[:, :ew],
                    mybir.ActivationFunctionType.Exp, scale=scale)
                if apply_tail:
                    for j, w, off in ((jA, wA, 0), (jB, wB, 2 * TILE)):
                        if not w:
                            continue
                        pL, pR = 2 * j, 2 * j + 1
                        if w == 2 * TILE and m >= counts[pR] - 4:
                            nc.vector.tensor_mul(
                                psb[:, off + TILE:off + 2 * TILE],
                                psb[:, off + TILE:off + 2 * TILE],
                                tm_sb[:, m - (counts[pR] - 4), :])
                        if m >= counts[pL] - 4:
                            nc.vector.tensor_mul(
                                psb[:, off:off + TILE],
                                psb[:, off:off + TILE],
                                tm_sb[:, m - (counts[pL] - 4), :])
                if wA:
                    pending.append((m, jA, wA, psb, 0))
                if wB:
                    pending.append((m, jB, wB, psb, 2 * TILE))
                flush_pending(5)

            # sweep B first: duo {2,3} (kv chunks < 16) rides waves 0-1;
            # projections of groups 0-3 and the deferred q duo-0 are filler
            push_proj_group(0)
            drain_fill(len(fill_q))
            stq0 = {}
            for cs in range(0, DC, 2):
                fill_q.append(
                    (lambda cs_: lambda: qproj_piece(0, cs_, cs_ + 2, stq0))(cs))
            nB = counts[4]
            for u, m in enumerate(range(nB)):
                if m % 4 == 0:
                    # hard boundary: everything pushed so far (incl. the
                    # previous group's projection) must be emitted before
                    # this group's units read khT/vh1
                    drain_fill(len(fill_q))
                    if m // 4 + 1 < 4:
                        push_proj_group(m // 4 + 1)
                if m == 4:
                    emit_wave_dma(2)
                if m == 8:
                    emit_wave_dma(3)
                drain_fill(-(-len(fill_q) // (nB - u)))
                emit_unit(1, m)
            flush_pending(0)

            # sweep A: duo {0,1} over all chunks; late groups' projections
            # and sweep-B finalization are the filler
            for m in range(NKV):
                if m in (0, 16, 20, 24, 28):
                    # hard boundary: pending finalize / projection writers
                    # must precede their readers in program order
                    drain_fill(len(fill_q))
                if m in (10, 12, 16, 20):
                    push_proj_group({10: 4, 12: 5, 16: 6, 20: 7}[m])
                rem = NKV - m
                drain_fill(-(-len(fill_q) // max(rem - 8, 1)))
                emit_unit(0, m)
            flush_pending(0)
            drain_fill(len(fill_q))
    nc.compile()
    return nc


def _get_program(key, counts, apply_tail):
    if key not in _CACHE:
        _CACHE[key] = _build_program(counts, apply_tail)
    return _CACHE[key]


def _numpy_fallback(q, k, v, mask, Wq, Wk, Wv):
    qh = q.astype(np.float32) @ Wq
    kh = k.astype(np.float32) @ Wk
    vh = v.astype(np.float32) @ Wv
    out = np.empty((B, T, H), np.float32)
    neg = np.float32(-1e30)
    for b in range(B):
        s = (qh[b] @ kh[b].T) / np.float32(np.sqrt(H))
        s = np.where(mask == 0, neg, s)
        s = s - s.max(axis=-1, keepdims=True)
        e = np.exp(s)
        w = e / e.sum(axis=-1, keepdims=True)
        out[b] = w @ vh[b]
    return out


def _pack_weights(Wq, Wk, Wv, np_in):
    # [D, H] -> [128, DC, H] with d = c*128 + p
    def pk(w):
        return w.reshape(DC, 128, H).transpose(1, 0, 2)
    wall = np.concatenate([pk(Wq), pk(Wk), pk(Wv)], axis=1)
    return np.ascontiguousarray(wall, np_in)


def _make_in_maps(q, k, v, mask, Wq, Wk, Wv, apply_tail):
    import ml_dtypes
    np_in = ml_dtypes.bfloat16

    wall = _pack_weights(Wq, Wk, Wv, np_in)
    in_maps = []
    metas = []
    # tail-mask blocks: keep iff (tq - p) >= c_s with c_s = 128*s - 256*half
    pp = np.arange(128)[:, None]
    tt = np.arange(TILE)[None, :]
    for c in range(8):
        b, h = divmod(c, 2)
        tiles = TILES_H0 if h == 0 else TILES_H1
        qT_slab = np.concatenate(
            [q[b, i * TILE:(i + 1) * TILE, :].T for i in tiles], axis=1)
        im = {
            "qT": np.ascontiguousarray(qT_slab, np_in),
            "kT": np.ascontiguousarray(k[b].T, np_in),
            "vT": np.ascontiguousarray(v[b].T, np_in),
            "wall": wall,
        }
        if apply_tail:
            tmask = np.zeros((128, 4, TILE), np.float32)
            for s in range(4):
                c_s = 128 * s - 256 * h
                tmask[:, s, :] = ((tt - pp) >= c_s).astype(np.float32)
            im["tmask"] = np.ascontiguousarray(tmask, np_in)
        in_maps.append(im)
        metas.append((b, tiles))
    return in_maps, metas


def kernel(q, k, v, mask, Wq, Wk, Wv):
    from concourse.bass_utils import run_bass_kernel_spmd

    q = np.ascontiguousarray(q, np.float32)
    k = np.ascontiguousarray(k, np.float32)
    v = np.ascontiguousarray(v, np.float32)
    Wq = np.ascontiguousarray(Wq, np.float32)
    Wk = np.ascontiguousarray(Wk, np.float32)
    Wv = np.ascontiguousarray(Wv, np.float32)
    mask = np.asarray(mask)

    is_tril = bool((mask == np.tril(np.ones((T, T), mask.dtype))).all())
    is_ones = bool((mask == 1).all())
    if not (is_tril or is_ones):
        return _numpy_fallback(q, k, v, mask, Wq, Wk, Wv)

    counts = COUNTS if is_tril else [NKV] * NPOS
    apply_tail = is_tril
    nc = _get_program(("v10b", is_tril), counts, apply_tail)

    in_maps, metas = _make_in_maps(q, k, v, mask, Wq, Wk, Wv, apply_tail)
    res = run_bass_kernel_spmd(nc, in_maps, list(range(8)))

    out = np.empty((B, T, H), np.float32)
    for c in range(8):
        b, tiles = metas[c]
        oc = res.results[c]["out"]
        for p, i in enumerate(tiles):
            out[b, i * TILE:(i + 1) * TILE, :] = oc[p * TILE:(p + 1) * TILE, :]
    return out


# revision 18
# speedup vs baseline: 1.1329x; 1.1329x over previous
"""Trainium2 Bass kernel for single-head causal attention with projections.

Reference computation (B=4, T=4096, D=1024, H=64):
    qh = q @ Wq; kh = k @ Wk; vh = v @ Wv          # [B,T,H]
    S  = qh @ kh.T / sqrt(H)  (causal masked)       # [B,T,T]
    out = softmax(S) @ vh                           # [B,T,H]

Sharding: 8 cores = 4 batches x 2 query-halves. Each core owns one batch's
full K/V and 8 query tiles of 256 rows, chosen by folded pairing so causal
work is balanced; a position-padded schedule makes all 8 cores run one
identical SPMD program (per-core differences live entirely in the data).

Final (v9) design notes:
  - bf16 everywhere on-chip; scores stay in ST orientation (tk on
    partitions): exp(S^T) feeds the PV matmul directly and an appended ones
    column in vh gives the softmax denominator for free.  No running max:
    scores are O(5), exp is safely in fp32 range.
  - All input DMAs are 2D [128, 1024] APs on ONE queue in need-order
    (q, then k/v per wave) so the critical path is never bandwidth-shared.
  - PE 32x32 tiling packs pairs of matmuls to run concurrently:
    k-proj (cols 0-63) || v-proj (cols 64-127) halve projection time, and
    scores for chunks (m, m+1) run as row-tiles (rows 0-63 / 64-127)
    against khT/qhT copies duplicated onto partitions 64-127 (SBUF-to-SBUF
    DMA, no HBM cost).  Attention runs as 4 per-pair passes over the
    resident khT/vh1, one [128,1024] double score tile + one exp per chunk
    pair (40 ACTIVATEs total).
  - Projection matmuls of the NEXT kv group and lazy pair finalization are
    interleaved as PE filler between attention units so the in-order PE
    never idles and the HAM clock-gate keeps the array at 2.4 GHz.
  - Causal tail masks collapse to 4 shared [128,256] blocks per core (the
    block depends only on 128*s - 256*half), applied on DVE.
"""

import collections

import numpy as np

B, T, D, H = 4, 4096, 1024, 64
TILE = 256          # tq position tile
GROUP = 512         # kv / projection t-group
NPOS = 8            # q position tiles per core
DC = D // 128       # d chunks
NKV = T // 128      # kv chunks
NG = T // GROUP     # kv groups (8)
TQ = NPOS * TILE    # q rows per core (2048)
QG = TQ // GROUP    # q groups (4)
NPAIR = NPOS // 2   # position pairs (4)

# per-position kv chunk counts (identical across cores): 32,28,...,4
COUNTS = [NKV - 4 * p for p in range(NPOS)]
# tile indices owned by a core: half 0 -> even tiles, half 1 -> odd tiles
TILES_H0 = [14 - 2 * p for p in range(NPOS)]
TILES_H1 = [15 - 2 * p for p in range(NPOS)]

_CACHE = {}


def _build_program(counts, apply_tail):
    import concourse.bacc as bacc
    import concourse.mybir as mybir
    import concourse.tile as tile
    from concourse.masks import make_identity

    f32 = mybir.dt.float32
    bf16 = mybir.dt.bfloat16

    nc = bacc.Bacc(None, target_bir_lowering=False, debug=False)
    qT = nc.declare_dram_parameter("qT", [D, TQ], bf16, isOutput=False)
    kT = nc.declare_dram_parameter("kT", [D, T], bf16, isOutput=False)
    vT = nc.declare_dram_parameter("vT", [D, T], bf16, isOutput=False)
    # packed [128, (3 tensors x 8 chunks), H]: partition-contiguous layout
    wall = nc.declare_dram_parameter("wall", [128, 3 * DC, H], bf16,
                                     isOutput=False)
    if apply_tail:
        tmask = nc.declare_dram_parameter(
            "tmask", [128, 4, TILE], bf16, isOutput=False)
    out = nc.declare_dram_parameter("out", [TQ, H], f32, isOutput=True)

    qT_r = qT.rearrange("(c p) t -> p c t", p=128)
    kT_r = kT.rearrange("(c p) t -> p c t", p=128)
    vT_r = vT.rearrange("(c p) t -> p c t", p=128)
    out_r = out.rearrange("(s p) h -> p s h", p=128)
    scale = 1.0 / float(np.sqrt(H))

    with tile.TileContext(nc) as tc:
        with (
            tc.tile_pool(name="singles", bufs=1) as singles,
            tc.tile_pool(name="qstage", bufs=1) as qstage,
            tc.tile_pool(name="kring", bufs=4) as kring,
            tc.tile_pool(name="vring", bufs=4) as vring,
            tc.tile_pool(name="work", bufs=6) as work,
            tc.tile_pool(name="small", bufs=3) as small,
            tc.tile_pool(name="proj_ps", bufs=2, space="PSUM") as pps,
            tc.tile_pool(name="st_ps", bufs=2, space="PSUM") as stps,
            tc.tile_pool(name="pvt_ps", bufs=1, space="PSUM") as pvtps,
        ):
            # ---- constants / staging (DMA issue order matters) ----
            w_sb = singles.tile([128, 3 * DC, H], bf16, tag="wall")
            nc.sync.dma_start(out=w_sb, in_=wall[:, :, :])
            wq_sb = w_sb[:, 0 * DC:1 * DC, :]
            wk_sb = w_sb[:, 1 * DC:2 * DC, :]
            wv_sb = w_sb[:, 2 * DC:3 * DC, :]
            tm_sb = None
            if apply_tail:
                tm_sb = singles.tile([128, 4, TILE], bf16, tag="tm")
                nc.sync.dma_start(out=tm_sb, in_=tmask[:, :, :])

            # qT resident; 2D DMAs per (chunk, half); half 1 is issued
            # after kv wave 0 (sweep A only needs half 0 early)
            qT_sb = qstage.tile([128, DC, TQ], bf16, tag="qt")

            def emit_q_dma(hh):
                sl = slice(hh * TQ // 2, (hh + 1) * TQ // 2)
                for c in range(DC):
                    nc.sync.dma_start(out=qT_sb[:, c, sl],
                                      in_=qT_r[:, c, sl])

            emit_q_dma(0)

            ident_b = singles.tile([H, H], bf16, tag="idb")
            make_identity(nc, ident_b)
            ident_f = singles.tile([H + 1, H + 1], f32, tag="idf")
            make_identity(nc, ident_f)
            # identity copy on partitions 64-127 for hi-half transposes
            ident_hi = singles.tile([128, H], bf16, tag="idh")
            nc.gpsimd.dma_start(out=ident_hi[H:128, :], in_=ident_b)

            qhT = singles.tile([H, TQ], bf16, tag="qhT")
            khT = singles.tile([H, T], bf16, tag="khT")
            vh1 = singles.tile([128, NKV, H + 1], bf16, tag="vh1")
            nc.vector.memset(vh1[:, :, H:H + 1], 1.0)
            out_sb = singles.tile([128, TQ // 128, H], f32, tag="osb")

            kv_tiles = {}

            def emit_wave_dma(w):
                ksl = slice(w * 2 * GROUP, (w + 1) * 2 * GROUP)
                kt = kring.tile([128, DC, 2 * GROUP], bf16, tag="kt")
                vt = vring.tile([128, DC, 2 * GROUP], bf16, tag="vt")
                for c in range(DC):
                    nc.sync.dma_start(out=kt[:, c, :], in_=kT_r[:, c, ksl])
                    nc.sync.dma_start(out=vt[:, c, :], in_=vT_r[:, c, ksl])
                kv_tiles[w] = (kt, vt)

            emit_wave_dma(0)
            emit_q_dma(1)
            emit_wave_dma(1)

            # ---- q projection (col-tiled pairs); duo 0 upfront, duo 1
            # deferred into the filler queue (sweep A only needs cols 0-1024)
            def qproj_piece(qp, cs, ce, st):
                g0, g1 = 2 * qp, 2 * qp + 1
                s0 = slice(g0 * GROUP, (g0 + 1) * GROUP)
                s1 = slice(g1 * GROUP, (g1 + 1) * GROUP)
                if cs == 0:
                    st["pha"] = pps.tile([128, GROUP], f32, tag="ph",
                                         name="phqa")
                    st["phb"] = pps.tile([128, GROUP], f32, tag="ph",
                                         name="phqb")
                for c in range(cs, ce):
                    nc.tensor.matmul(st["pha"][0:H, :], wq_sb[:, c, :],
                                     qT_sb[:, c, s0],
                                     start=(c == 0), stop=(c == DC - 1),
                                     tile_position=(0, 0))
                    nc.tensor.matmul(st["phb"][H:128, :], wq_sb[:, c, :],
                                     qT_sb[:, c, s1],
                                     start=(c == 0), stop=(c == DC - 1),
                                     tile_position=(0, 64))
                if ce == DC:
                    nc.vector.tensor_copy(qhT[:, s0], st["pha"][0:H, :])
                    nc.vector.tensor_copy(qhT[:, s1], st["phb"][H:128, :])

            stq0 = {}
            for cs in range(0, DC, 2):
                qproj_piece(0, cs, cs + 2, stq0)

            # ---- filler machinery: small PE pieces run between attn units --
            fill_q = collections.deque()

            def drain_fill(n):
                for _ in range(min(n, len(fill_q))):
                    fill_q.popleft()()

            def push_proj_group(g):
                w, half = divmod(g, 2)
                kt, vt = kv_tiles[w]
                hsl = slice(half * GROUP, (half + 1) * GROUP)
                gsl = slice(g * GROUP, (g + 1) * GROUP)
                st = {}

                def kvproj(cs, ce):
                    def f():
                        if cs == 0:
                            st["phk"] = pps.tile([128, GROUP], f32, tag="ph",
                                                 name="phk")
                            st["phv"] = pps.tile([128, GROUP], f32, tag="ph",
                                                 name="phv")
                        for c in range(cs, ce):
                            nc.tensor.matmul(
                                st["phk"][0:H, :], wk_sb[:, c, :],
                                kt[:, c, hsl],
                                start=(c == 0), stop=(c == DC - 1),
                                tile_position=(0, 0))
                            nc.tensor.matmul(
                                st["phv"][H:128, :], wv_sb[:, c, :],
                                vt[:, c, hsl],
                                start=(c == 0), stop=(c == DC - 1),
                                tile_position=(0, 64))
                        if ce == DC:
                            nc.vector.tensor_copy(khT[:, gsl],
                                                  st["phk"][0:H, :])
                            st["vtmp"] = small.tile(
                                [128, GROUP], bf16, tag="vtmp", name="vtmp")
                            nc.vector.tensor_copy(st["vtmp"][H:128, :],
                                                  st["phv"][H:128, :])
                    return f

                def vtrans(s):
                    def f():
                        tr = pps.tile([128, H], bf16, tag="ph")
                        nc.tensor.transpose(
                            tr, st["vtmp"][H:128, s * 128:(s + 1) * 128],
                            ident_hi[H:128, :], tile_position=(64, 0))
                        nc.vector.tensor_copy(vh1[:, g * 4 + s, 0:H], tr)
                    return f

                for cs in range(0, DC, 2):
                    fill_q.append(kvproj(cs, cs + 2))
                for s in range(GROUP // 128):
                    fill_q.append(vtrans(s))

            def push_finalize(j):
                st = {}

                def fcopy():
                    st["pvt_sb"] = small.tile([H + 1, 2 * TILE], f32,
                                              tag="pvtsb", name="pvtsb")
                    nc.vector.tensor_copy(st["pvt_sb"], pvt[:, j % 2, :])

                def fnorm(s):
                    def f():
                        tr = pps.tile([128, H + 1], f32, tag="ph")
                        nc.tensor.transpose(
                            tr, st["pvt_sb"][:, s * 128:(s + 1) * 128],
                            ident_f)
                        ofull = small.tile([128, H + 1], f32, tag="of")
                        nc.vector.tensor_copy(ofull, tr)
                        rec = small.tile([128, 1], f32, tag="rec")
                        nc.vector.reciprocal(rec, ofull[:, H:H + 1])
                        nc.vector.tensor_scalar_mul(
                            out_sb[:, j * 4 + s, :], ofull[:, :H], rec)
                    return f

                def fout():
                    nc.gpsimd.dma_start(
                        out=out_r[:, j * 4:(j + 1) * 4, :],
                        in_=out_sb[:, j * 4:(j + 1) * 4, :])

                fill_q.append(fcopy)
                for s in range(2 * TILE // 128):
                    fill_q.append(fnorm(s))
                fill_q.append(fout)

            # ---- attention: 4 per-pair passes over resident khT/vh1 ----
            pvt = pvtps.tile([H + 1, 2, 2 * TILE], f32, tag="pvt")
            pending = []

            def flush_pending(limit):
                while len(pending) > limit:
                    m, j, width, psb, off = pending.pop(0)
                    last = m == counts[2 * j] - 1
                    nc.tensor.matmul(
                        pvt[:, j % 2, :width], vh1[:, m, :],
                        psb[:, off:off + width],
                        start=(m == 0), stop=last,
                        skip_group_check=True)
                    if last:
                        push_finalize(j)

            def emit_unit(duo, m):
                jA, jB = 2 * duo, 2 * duo + 1
                wA = 0
                if counts[2 * jA] > m:
                    wA = 2 * TILE if counts[2 * jA + 1] > m else TILE
                wB = 0
                if counts[2 * jB] > m:
                    wB = 2 * TILE if counts[2 * jB + 1] > m else TILE
                if wA == 0 and wB == 0:
                    return
                kh = khT[:, m * 128:(m + 1) * 128]
                stp = stps.tile([128, 4 * TILE], f32, tag="st")
                if wA:
                    nc.tensor.matmul(
                        stp[:, :wA], kh,
                        qhT[:, jA * 2 * TILE:jA * 2 * TILE + wA],
                        start=True, stop=True)
                if wB:
                    nc.tensor.matmul(
                        stp[:, 2 * TILE:2 * TILE + wB], kh,
                        qhT[:, jB * 2 * TILE:jB * 2 * TILE + wB],
                        start=True, stop=True)
                ew = (2 * TILE + wB) if wB else wA
                psb = work.tile([128, 4 * TILE], bf16, tag="p")
                nc.scalar.activation(
                    psb[:, :ew], stp[:, :ew],
                    mybir.ActivationFunctionType.Exp, scale=scale)
                if apply_tail:
                    for j, w, off in ((jA, wA, 0), (jB, wB, 2 * TILE)):
                        if not w:
                            continue
                        pL, pR = 2 * j, 2 * j + 1
                        if w == 2 * TILE and m >= counts[pR] - 4:
                            nc.vector.tensor_mul(
                                psb[:, off + TILE:off + 2 * TILE],
                                psb[:, off + TILE:off + 2 * TILE],
                                tm_sb[:, m - (counts[pR] - 4), :])
                        if m >= counts[pL] - 4:
                            nc.vector.tensor_mul(
                                psb[:, off:off + TILE],
                                psb[:, off:off + TILE],
                                tm_sb[:, m - (counts[pL] - 4), :])
                if wA:
                    pending.append((m, jA, wA, psb, 0))
                if wB:
                    pending.append((m, jB, wB, psb, 2 * TILE))
                flush_pending(5)

            # sweep A: duo {0,1} during the kv stream; next group's
            # projection (and the deferred q duo-1 projection) is the filler
            push_proj_group(0)
            drain_fill(len(fill_q))
            stq1 = {}
            for cs in range(0, DC, 2):
                fill_q.append(
                    (lambda cs_: lambda: qproj_piece(1, cs_, cs_ + 2, stq1))(cs))
            for g in range(NG):
                if g + 1 < NG:
                    push_proj_group(g + 1)
                if g % 2 == 0 and g // 2 + 2 < NG // 2:
                    emit_wave_dma(g // 2 + 2)
                ms = list(range(4 * g, 4 * g + 4))
                for u, m in enumerate(ms):
                    drain_fill(-(-len(fill_q) // (len(ms) - u)))
                    emit_unit(0, m)
                drain_fill(len(fill_q))
            flush_pending(0)

            # sweep B: duo {2,3} re-streams resident khT/vh1; finalize of
            # sweep-A pairs is the filler
            for u, m in enumerate(range(counts[4])):
                drain_fill(-(-len(fill_q) // (counts[4] - u)))
                emit_unit(1, m)
            flush_pending(0)
            drain_fill(len(fill_q))
    nc.compile()
    return nc


def _get_program(key, counts, apply_tail):
    if key not in _CACHE:
        _CACHE[key] = _build_program(counts, apply_tail)
    return _CACHE[key]


def _numpy_fallback(q, k, v, mask, Wq, Wk, Wv):
    qh = q.astype(np.float32) @ Wq
    kh = k.astype(np.float32) @ Wk
    vh = v.astype(np.float32) @ Wv
    out = np.empty((B, T, H), np.float32)
    neg = np.float32(-1e30)
    for b in range(B):
        s = (qh[b] @ kh[b].T) / np.float32(np.sqrt(H))
        s = np.where(mask == 0, neg, s)
        s = s - s.max(axis=-1, keepdims=True)
        e = np.exp(s)
        w = e / e.sum(axis=-1, keepdims=True)
        out[b] = w @ vh[b]
    return out


def _pack_weights(Wq, Wk, Wv, np_in):
    # [D, H] -> [128, DC, H] with d = c*128 + p
    def pk(w):
        return w.reshape(DC, 128, H).transpose(1, 0, 2)
    wall = np.concatenate([pk(Wq), pk(Wk), pk(Wv)], axis=1)
    return np.ascontiguousarray(wall, np_in)


def _make_in_maps(q, k, v, mask, Wq, Wk, Wv, apply_tail):
    import ml_dtypes
    np_in = ml_dtypes.bfloat16

    wall = _pack_weights(Wq, Wk, Wv, np_in)
    in_maps = []
    metas = []
    # tail-mask blocks: keep iff (tq - p) >= c_s with c_s = 128*s - 256*half
    pp = np.arange(128)[:, None]
    tt = np.arange(TILE)[None, :]
    for c in range(8):
        b, h = divmod(c, 2)
        tiles = TILES_H0 if h == 0 else TILES_H1
        qT_slab = np.concatenate(
            [q[b, i * TILE:(i + 1) * TILE, :].T for i in tiles], axis=1)
        im = {
            "qT": np.ascontiguousarray(qT_slab, np_in),
            "kT": np.ascontiguousarray(k[b].T, np_in),
            "vT": np.ascontiguousarray(v[b].T, np_in),
            "wall": wall,
        }
        if apply_tail:
            tmask = np.zeros((128, 4, TILE), np.float32)
            for s in range(4):
                c_s = 128 * s - 256 * h
                tmask[:, s, :] = ((tt - pp) >= c_s).astype(np.float32)
            im["tmask"] = np.ascontiguousarray(tmask, np_in)
        in_maps.append(im)
        metas.append((b, tiles))
    return in_maps, metas


def kernel(q, k, v, mask, Wq, Wk, Wv):
    from concourse.bass_utils import run_bass_kernel_spmd

    q = np.ascontiguousarray(q, np.float32)
    k = np.ascontiguousarray(k, np.float32)
    v = np.ascontiguousarray(v, np.float32)
    Wq = np.ascontiguousarray(Wq, np.float32)
    Wk = np.ascontiguousarray(Wk, np.float32)
    Wv = np.ascontiguousarray(Wv, np.float32)
    mask = np.asarray(mask)

    is_tril = bool((mask == np.tril(np.ones((T, T), mask.dtype))).all())
    is_ones = bool((mask == 1).all())
    if not (is_tril or is_ones):
        return _numpy_fallback(q, k, v, mask, Wq, Wk, Wv)

    counts = COUNTS if is_tril else [NKV] * NPOS
    apply_tail = is_tril
    nc = _get_program(("v9b", is_tril), counts, apply_tail)

    in_maps, metas = _make_in_maps(q, k, v, mask, Wq, Wk, Wv, apply_tail)
    res = run_bass_kernel_spmd(nc, in_maps, list(range(8)))

    out = np.empty((B, T, H), np.float32)
    for c in range(8):
        b, tiles = metas[c]
        oc = res.results[c]["out"]
        for p, i in enumerate(tiles):
            out[b, i * TILE:(i + 1) * TILE, :] = oc[p * TILE:(p + 1) * TILE, :]
    return out


# revision 19
# speedup vs baseline: 1.1972x; 1.0567x over previous
"""Trainium2 Bass kernel for single-head causal attention with projections.

Reference computation (B=4, T=4096, D=1024, H=64):
    qh = q @ Wq; kh = k @ Wk; vh = v @ Wv          # [B,T,H]
    S  = qh @ kh.T / sqrt(H)  (causal masked)       # [B,T,T]
    out = softmax(S) @ vh                           # [B,T,H]

Sharding: 8 cores = 4 batches x 2 query-halves. Each core owns one batch's
full K/V and 8 query tiles of 256 rows, chosen by folded pairing so causal
work is balanced; a position-padded schedule makes all 8 cores run one
identical SPMD program (per-core differences live entirely in the data).

Final (v9) design notes:
  - bf16 everywhere on-chip; scores stay in ST orientation (tk on
    partitions): exp(S^T) feeds the PV matmul directly and an appended ones
    column in vh gives the softmax denominator for free.  No running max:
    scores are O(5), exp is safely in fp32 range.
  - All input DMAs are 2D [128, 1024] APs on ONE queue in need-order
    (q half-0, kv wave 0, q half-1, waves 1-3 with k/v interleaved per
    chunk) so the critical path is never bandwidth-shared.
  - PE 32x32 tiling packs pairs of matmuls to run concurrently:
    k-proj (cols 0-63) || v-proj (cols 64-127) halve projection time, and
    k-proj and v-proj chains accumulate in separate PSUM banks so the
    bank-overlap tracker never serializes their copy-outs.  Attention runs
    as two sweeps over the resident khT/vh1 (pair duo {0,1} riding the kv
    stream, then {2,3}), one [128,1024] double score tile + one exp per
    duo-chunk (48 ACTIVATEs total).
  - Projection matmuls of the NEXT kv group and lazy pair finalization are
    interleaved as PE filler between attention units so the in-order PE
    never idles and the HAM clock-gate keeps the array at 2.4 GHz.
  - Causal tail masks collapse to 4 shared [128,256] blocks per core (the
    block depends only on 128*s - 256*half), applied on DVE.
"""

import collections

import numpy as np

B, T, D, H = 4, 4096, 1024, 64
TILE = 256          # tq position tile
GROUP = 512         # kv / projection t-group
NPOS = 8            # q position tiles per core
DC = D // 128       # d chunks
NKV = T // 128      # kv chunks
NG = T // GROUP     # kv groups (8)
TQ = NPOS * TILE    # q rows per core (2048)
QG = TQ // GROUP    # q groups (4)
NPAIR = NPOS // 2   # position pairs (4)

# per-position kv chunk counts (identical across cores): 32,28,...,4
COUNTS = [NKV - 4 * p for p in range(NPOS)]
# tile indices owned by a core: half 0 -> even tiles, half 1 -> odd tiles
TILES_H0 = [14 - 2 * p for p in range(NPOS)]
TILES_H1 = [15 - 2 * p for p in range(NPOS)]

_CACHE = {}


def _build_program(counts, apply_tail):
    import concourse.bacc as bacc
    import concourse.mybir as mybir
    import concourse.tile as tile
    from concourse.masks import make_identity

    f32 = mybir.dt.float32
    bf16 = mybir.dt.bfloat16

    nc = bacc.Bacc(None, target_bir_lowering=False, debug=False)
    qT = nc.declare_dram_parameter("qT", [D, TQ], bf16, isOutput=False)
    kT = nc.declare_dram_parameter("kT", [D, T], bf16, isOutput=False)
    vT = nc.declare_dram_parameter("vT", [D, T], bf16, isOutput=False)
    # packed [128, (3 tensors x 8 chunks), H]: partition-contiguous layout
    wall = nc.declare_dram_parameter("wall", [128, 3 * DC, H], bf16,
                                     isOutput=False)
    if apply_tail:
        tmask = nc.declare_dram_parameter(
            "tmask", [128, 4, TILE], bf16, isOutput=False)
    out = nc.declare_dram_parameter("out", [TQ, H], f32, isOutput=True)

    qT_r = qT.rearrange("(c p) t -> p c t", p=128)
    kT_r = kT.rearrange("(c p) t -> p c t", p=128)
    vT_r = vT.rearrange("(c p) t -> p c t", p=128)
    out_r = out.rearrange("(s p) h -> p s h", p=128)
    scale = 1.0 / float(np.sqrt(H))

    with tile.TileContext(nc) as tc:
        with (
            tc.tile_pool(name="singles", bufs=1) as singles,
            tc.tile_pool(name="qstage", bufs=1) as qstage,
            tc.tile_pool(name="kring", bufs=4) as kring,
            tc.tile_pool(name="vring", bufs=4) as vring,
            tc.tile_pool(name="work", bufs=6) as work,
            tc.tile_pool(name="small", bufs=3) as small,
            tc.tile_pool(name="proj_ps", bufs=2, space="PSUM") as pps,
            tc.tile_pool(name="st_ps", bufs=2, space="PSUM") as stps,
            tc.tile_pool(name="pvt_ps", bufs=1, space="PSUM") as pvtps,
        ):
            # ---- constants / staging (DMA issue order matters) ----
            w_sb = singles.tile([128, 3 * DC, H], bf16, tag="wall")
            nc.sync.dma_start(out=w_sb, in_=wall[:, :, :])
            wq_sb = w_sb[:, 0 * DC:1 * DC, :]
            wk_sb = w_sb[:, 1 * DC:2 * DC, :]
            wv_sb = w_sb[:, 2 * DC:3 * DC, :]
            tm_sb = None
            if apply_tail:
                tm_sb = singles.tile([128, 4, TILE], bf16, tag="tm")
                nc.sync.dma_start(out=tm_sb, in_=tmask[:, :, :])

            # qT resident; 2D DMAs per (chunk, half); half 1 is issued
            # after kv wave 0 (sweep A only needs half 0 early)
            qT_sb = qstage.tile([128, DC, TQ], bf16, tag="qt")

            def emit_q_dma(hh):
                sl = slice(hh * TQ // 2, (hh + 1) * TQ // 2)
                for c in range(DC):
                    nc.sync.dma_start(out=qT_sb[:, c, sl],
                                      in_=qT_r[:, c, sl])

            emit_q_dma(0)

            ident_b = singles.tile([H, H], bf16, tag="idb")
            make_identity(nc, ident_b)
            ident_f = singles.tile([H + 1, H + 1], f32, tag="idf")
            make_identity(nc, ident_f)
            # identity copy on partitions 64-127 for hi-half transposes
            ident_hi = singles.tile([128, H], bf16, tag="idh")
            nc.gpsimd.dma_start(out=ident_hi[H:128, :], in_=ident_b)

            qhT = singles.tile([H, TQ], bf16, tag="qhT")
            khT = singles.tile([H, T], bf16, tag="khT")
            vh1 = singles.tile([128, NKV, H + 1], bf16, tag="vh1")
            nc.vector.memset(vh1[:, :, H:H + 1], 1.0)
            out_sb = singles.tile([128, TQ // 128, H], f32, tag="osb")

            kv_tiles = {}

            def emit_wave_dma(w):
                ksl = slice(w * 2 * GROUP, (w + 1) * 2 * GROUP)
                kt = kring.tile([128, DC, 2 * GROUP], bf16, tag="kt")
                vt = vring.tile([128, DC, 2 * GROUP], bf16, tag="vt")
                for c in range(DC):
                    nc.sync.dma_start(out=kt[:, c, :], in_=kT_r[:, c, ksl])
                    nc.sync.dma_start(out=vt[:, c, :], in_=vT_r[:, c, ksl])
                kv_tiles[w] = (kt, vt)

            emit_wave_dma(0)
            emit_q_dma(1)
            emit_wave_dma(1)

            # ---- q projection (col-tiled pairs); duo 0 upfront, duo 1
            # deferred into the filler queue (sweep A only needs cols 0-1024)
            def qproj_piece(qp, cs, ce, st):
                g0, g1 = 2 * qp, 2 * qp + 1
                s0 = slice(g0 * GROUP, (g0 + 1) * GROUP)
                s1 = slice(g1 * GROUP, (g1 + 1) * GROUP)
                if cs == 0:
                    st["pha"] = pps.tile([128, GROUP], f32, tag="ph",
                                         name="phqa")
                    st["phb"] = pps.tile([128, GROUP], f32, tag="ph",
                                         name="phqb")
                for c in range(cs, ce):
                    nc.tensor.matmul(st["pha"][0:H, :], wq_sb[:, c, :],
                                     qT_sb[:, c, s0],
                                     start=(c == 0), stop=(c == DC - 1),
                                     tile_position=(0, 0))
                    nc.tensor.matmul(st["phb"][H:128, :], wq_sb[:, c, :],
                                     qT_sb[:, c, s1],
                                     start=(c == 0), stop=(c == DC - 1),
                                     tile_position=(0, 64))
                if ce == DC:
                    nc.vector.tensor_copy(qhT[:, s0], st["pha"][0:H, :])
                    nc.vector.tensor_copy(qhT[:, s1], st["phb"][H:128, :])

            stq0 = {}
            for cs in range(0, DC, 2):
                qproj_piece(0, cs, cs + 2, stq0)

            # ---- filler machinery: small PE pieces run between attn units --
            fill_q = collections.deque()

            def drain_fill(n):
                for _ in range(min(n, len(fill_q))):
                    fill_q.popleft()()

            def push_proj_group(g):
                w, half = divmod(g, 2)
                kt, vt = kv_tiles[w]
                hsl = slice(half * GROUP, (half + 1) * GROUP)
                gsl = slice(g * GROUP, (g + 1) * GROUP)
                st = {}

                def kvproj(cs, ce):
                    def f():
                        if cs == 0:
                            st["phk"] = pps.tile([128, GROUP], f32, tag="ph",
                                                 name="phk")
                            st["phv"] = pps.tile([128, GROUP], f32, tag="ph",
                                                 name="phv")
                        for c in range(cs, ce):
                            nc.tensor.matmul(
                                st["phk"][0:H, :], wk_sb[:, c, :],
                                kt[:, c, hsl],
                                start=(c == 0), stop=(c == DC - 1),
                                tile_position=(0, 0))
                            nc.tensor.matmul(
                                st["phv"][H:128, :], wv_sb[:, c, :],
                                vt[:, c, hsl],
                                start=(c == 0), stop=(c == DC - 1),
                                tile_position=(0, 64))
                        if ce == DC:
                            nc.vector.tensor_copy(khT[:, gsl],
                                                  st["phk"][0:H, :])
                            st["vtmp"] = small.tile(
                                [128, GROUP], bf16, tag="vtmp", name="vtmp")
                            nc.vector.tensor_copy(st["vtmp"][H:128, :],
                                                  st["phv"][H:128, :])
                    return f

                def vtrans(s):
                    def f():
                        tr = pps.tile([128, H], bf16, tag="ph")
                        nc.tensor.transpose(
                            tr, st["vtmp"][H:128, s * 128:(s + 1) * 128],
                            ident_hi[H:128, :], tile_position=(64, 0))
                        nc.vector.tensor_copy(vh1[:, g * 4 + s, 0:H], tr)
                    return f

                for cs in range(0, DC, 2):
                    fill_q.append(kvproj(cs, cs + 2))
                for s in range(GROUP // 128):
                    fill_q.append(vtrans(s))

            def push_finalize(j):
                st = {}

                def fcopy():
                    st["pvt_sb"] = small.tile([H + 1, 2 * TILE], f32,
                                              tag="pvtsb", name="pvtsb")
                    nc.vector.tensor_copy(st["pvt_sb"], pvt[:, j % 2, :])

                def fnorm(s):
                    def f():
                        tr = pps.tile([128, H + 1], f32, tag="ph")
                        nc.tensor.transpose(
                            tr, st["pvt_sb"][:, s * 128:(s + 1) * 128],
                            ident_f)
                        ofull = small.tile([128, H + 1], f32, tag="of")
                        nc.vector.tensor_copy(ofull, tr)
                        rec = small.tile([128, 1], f32, tag="rec")
                        nc.vector.reciprocal(rec, ofull[:, H:H + 1])
                        nc.vector.tensor_scalar_mul(
                            out_sb[:, j * 4 + s, :], ofull[:, :H], rec)
                    return f

                def fout():
                    nc.gpsimd.dma_start(
                        out=out_r[:, j * 4:(j + 1) * 4, :],
                        in_=out_sb[:, j * 4:(j + 1) * 4, :])

                fill_q.append(fcopy)
                for s in range(2 * TILE // 128):
                    fill_q.append(fnorm(s))
                fill_q.append(fout)

            # ---- attention: 4 per-pair passes over resident khT/vh1 ----
            pvt = pvtps.tile([H + 1, 2, 2 * TILE], f32, tag="pvt")
            pending = []

            def flush_pending(limit):
                while len(pending) > limit:
                    m, j, width, psb, off = pending.pop(0)
                    last = m == counts[2 * j] - 1
                    nc.tensor.matmul(
                        pvt[:, j % 2, :width], vh1[:, m, :],
                        psb[:, off:off + width],
                        start=(m == 0), stop=last,
                        skip_group_check=True)
                    if last:
                        push_finalize(j)

            def emit_unit(duo, m):
                jA, jB = 2 * duo, 2 * duo + 1
                wA = 0
                if counts[2 * jA] > m:
                    wA = 2 * TILE if counts[2 * jA + 1] > m else TILE
                wB = 0
                if counts[2 * jB] > m:
                    wB = 2 * TILE if counts[2 * jB + 1] > m else TILE
                if wA == 0 and wB == 0:
                    return
                kh = khT[:, m * 128:(m + 1) * 128]
                stp = stps.tile([128, 4 * TILE], f32, tag="st")
                if wA:
                    nc.tensor.matmul(
                        stp[:, :wA], kh,
                        qhT[:, jA * 2 * TILE:jA * 2 * TILE + wA],
                        start=True, stop=True)
                if wB:
                    nc.tensor.matmul(
                        stp[:, 2 * TILE:2 * TILE + wB], kh,
                        qhT[:, jB * 2 * TILE:jB * 2 * TILE + wB],
                        start=True, stop=True)
                ew = (2 * TILE + wB) if wB else wA
                psb = work.tile([128, 4 * TILE], bf16, tag="p")
                nc.scalar.activation(
                    psb[:, :ew], stp[:, :ew],
                    mybir.ActivationFunctionType.Exp, scale=scale)
                if apply_tail:
                    for j, w, off in ((jA, wA, 0), (jB, wB, 2 * TILE)):
                        if not w:
                            continue
                        pL, pR = 2 * j, 2 * j + 1
                        if w == 2 * TILE and m >= counts[pR] - 4:
                            nc.vector.tensor_mul(
                                psb[:, off + TILE:off + 2 * TILE],
                                psb[:, off + TILE:off + 2 * TILE],
                                tm_sb[:, m - (counts[pR] - 4), :])
                        if m >= counts[pL] - 4:
                            nc.vector.tensor_mul(
                                psb[:, off:off + TILE],
                                psb[:, off:off + TILE],
                                tm_sb[:, m - (counts[pL] - 4), :])
                if wA:
                    pending.append((m, jA, wA, psb, 0))
                if wB:
                    pending.append((m, jB, wB, psb, 2 * TILE))
                flush_pending(5)

            # sweep A: duo {0,1} during the kv stream; next group's
            # projection (and the deferred q duo-1 projection) is the filler
            push_proj_group(0)
            drain_fill(len(fill_q))
            stq1 = {}
            for cs in range(0, DC, 2):
                fill_q.append(
                    (lambda cs_: lambda: qproj_piece(1, cs_, cs_ + 2, stq1))(cs))
            for g in range(NG):
                if g + 1 < NG:
                    push_proj_group(g + 1)
                if g % 2 == 0 and g // 2 + 2 < NG // 2:
                    emit_wave_dma(g // 2 + 2)
                ms = list(range(4 * g, 4 * g + 4))
                for u, m in enumerate(ms):
                    drain_fill(-(-len(fill_q) // (len(ms) - u)))
                    emit_unit(0, m)
                drain_fill(len(fill_q))
            flush_pending(0)

            # sweep B: duo {2,3} re-streams resident khT/vh1; finalize of
            # sweep-A pairs is the filler
            for u, m in enumerate(range(counts[4])):
                drain_fill(-(-len(fill_q) // (counts[4] - u)))
                emit_unit(1, m)
            flush_pending(0)
            drain_fill(len(fill_q))
    nc.compile()
    return nc


def _get_program(key, counts, apply_tail):
    if key not in _CACHE:
        _CACHE[key] = _build_program(counts, apply_tail)
    return _CACHE[key]


def _numpy_fallback(q, k, v, mask, Wq, Wk, Wv):
    qh = q.astype(np.float32) @ Wq
    kh = k.astype(np.float32) @ Wk
    vh = v.astype(np.float32) @ Wv
    out = np.empty((B, T, H), np.float32)
    neg = np.float32(-1e30)
    for b in range(B):
        s = (qh[b] @ kh[b].T) / np.float32(np.sqrt(H))
        s = np.where(mask == 0, neg, s)
        s = s - s.max(axis=-1, keepdims=True)
        e = np.exp(s)
        w = e / e.sum(axis=-1, keepdims=True)
        out[b] = w @ vh[b]
    return out


def _pack_weights(Wq, Wk, Wv, np_in):
    # [D, H] -> [128, DC, H] with d = c*128 + p
    def pk(w):
        return w.reshape(DC, 128, H).transpose(1, 0, 2)
    wall = np.concatenate([pk(Wq), pk(Wk), pk(Wv)], axis=1)
    return np.ascontiguousarray(wall, np_in)


def _make_in_maps(q, k, v, mask, Wq, Wk, Wv, apply_tail):
    import ml_dtypes
    np_in = ml_dtypes.bfloat16

    wall = _pack_weights(Wq, Wk, Wv, np_in)
    in_maps = []
    metas = []
    # tail-mask blocks: keep iff (tq - p) >= c_s with c_s = 128*s - 256*half
    pp = np.arange(128)[:, None]
    tt = np.arange(TILE)[None, :]
    for c in range(8):
        b, h = divmod(c, 2)
        tiles = TILES_H0 if h == 0 else TILES_H1
        qT_slab = np.concatenate(
            [q[b, i * TILE:(i + 1) * TILE, :].T for i in tiles], axis=1)
        im = {
            "qT": np.ascontiguousarray(qT_slab, np_in),
            "kT": np.ascontiguousarray(k[b].T, np_in),
            "vT": np.ascontiguousarray(v[b].T, np_in),
            "wall": wall,
        }
        if apply_tail:
            tmask = np.zeros((128, 4, TILE), np.float32)
            for s in range(4):
                c_s = 128 * s - 256 * h
                tmask[:, s, :] = ((tt - pp) >= c_s).astype(np.float32)
            im["tmask"] = np.ascontiguousarray(tmask, np_in)
        in_maps.append(im)
        metas.append((b, tiles))
    return in_maps, metas


def kernel(q, k, v, mask, Wq, Wk, Wv):
    from concourse.bass_utils import run_bass_kernel_spmd

    q = np.ascontiguousarray(q, np.float32)
    k = np.ascontiguousarray(k, np.float32)
    v = np.ascontiguousarray(v, np.float32)
    Wq = np.ascontiguousarray(Wq, np.float32)
    Wk = np.ascontiguousarray(Wk, np.float32)
    Wv = np.ascontiguousarray(Wv, np.float32)
    mask = np.asarray(mask)

    is_tril = bool((mask == np.tril(np.ones((T, T), mask.dtype))).all())
    is_ones = bool((mask == 1).all())
    if not (is_tril or is_ones):
        return _numpy_fallback(q, k, v, mask, Wq, Wk, Wv)

    counts = COUNTS if is_tril else [NKV] * NPOS
    apply_tail = is_tril
    nc = _get_program(("v9b", is_tril), counts, apply_tail)

    in_maps, metas = _make_in_maps(q, k, v, mask, Wq, Wk, Wv, apply_tail)
    res = run_bass_kernel_spmd(nc, in_maps, list(range(8)))

    out = np.empty((B, T, H), np.float32)
    for c in range(8):
        b, tiles = metas[c]
        oc = res.results[c]["out"]
        for p, i in enumerate(tiles):
            out[b, i * TILE:(i + 1) * TILE, :] = oc[p * TILE:(p + 1) * TILE, :]
    return out
